# revision 38
# baseline (speedup 1.0000x reference)
"""Trainium2 Bass kernel for nn_DisAttLayer (disentangled-attention bias MLP).

Math (reference):
    e[b,m,n,h,:] = concat(pe[m-n+S], bi[b,m], bj[b,n], ci[b,m], cj[b,n])  (96)
    h1 = relu(e @ w1[:, :, h])     (96->32, per head)
    h2 = relu(h1 @ w2[:, :, h])    (32->16)
    score[b,h,m,n] = h2 @ w3[:, h] (16->1)

Key factorization: layer 1 is linear in the concat, so
    h1pre[b,m,n,h,k] = Ap[m-n+S,h,k] + Arow[b,m,h,k] + Acol[b,n,h,k]
where Ap/Arow/Acol are tiny per-table transforms (computed on-device from the
raw embedding tables and w1).  With the free axis taken as n' = 255-n, the
relative-position gather Ap[m-n+S] becomes a contiguous slice of a 384-wide
table, so no gather is needed at all.  Only layers 2+3 touch the full
(B,S,S,H) volume.

Sharding: 8 cores = batch b (4) x query-half m (2).  All per-core variation
is moved into the input data (pre-shifted e_pos slice, per-core one-hot
index masks), so a single SPMD program serves all cores.

Host does layout only (transpose/reshape/zero-pad/slicing of raw inputs and
integer->one-hot relabeling of the index sequences); every floating-point
multiply/add of the model runs on device.
"""

import os
from contextlib import ExitStack

import numpy as np

import concourse.bacc as bacc
import concourse.bass as bass
import concourse.tile as tile
from concourse import mybir
from concourse.bass_utils import run_bass_kernel_spmd

S = 256
H = 8
B = 4
MH = 128          # m-values per core
VB = 11           # e_bi / e_bj rows  (N_MB + 1)
VC = 102          # e_ci / e_cj rows  (N_C + 2)
APW = MH + S      # 384: width of the per-core shifted e_pos slice

F32 = mybir.dt.float32
F16 = mybir.dt.float16
BF16 = mybir.dt.bfloat16

# engine routing for the per-(m, head-group) bias+relu blocks (structural:
# one op per m since the bias is a per-partition scalar).  Routed in whole
# 4-op blocks to minimize cross-engine semaphore traffic; DVE is ~2x faster
# per op than ACT but ACT has idle capacity.
TS_ROUTE = ("dve", "dve", "act", "dve", "dve", "act", "dve", "dve")
N_WARMUP_MM = 12


def _declare_io(nc):
    def inp(name, shape):
        return nc.dram_tensor(name, list(shape), F16, kind="ExternalInput").ap()

    ins = {
        "eposT": inp("eposT", (2, 128, APW)),       # [g][hh*32+d, r_local]
        "ebiT": inp("ebiT", (128, VB)),             # [h*16+d, v]
        "ebjT": inp("ebjT", (128, VB)),
        "eciT": inp("eciT", (128, VC)),
        "ecjT": inp("ecjT", (128, VC)),
        "w1pe_blk": inp("w1pe_blk", (2, 128, 128)),  # [g][hh*32+d, hh*32+k]
        "w1bi_blk": inp("w1bi_blk", (128, 256)),     # [h*16+d, h*32+k]
        "w1bj_blk": inp("w1bj_blk", (128, 256)),
        "w1ci_blk": inp("w1ci_blk", (128, 256)),
        "w1cj_blk": inp("w1cj_blk", (128, 256)),
        "oh_b_row": inp("oh_b_row", (VB, MH)),
        "oh_c_row": inp("oh_c_row", (VC, MH)),
        "oh_b_col": inp("oh_b_col", (VB, S)),        # n-reversed
        "oh_c_col": inp("oh_c_col", (VC, S)),
        "w2blk": inp("w2blk", (2, 128, 64)),         # [g][hh*32+k, hh*16+l]
        # 4 zero-padded variants; variant q maps head h -> out partition 32q+h
        "w3blk4": inp("w3blk4", (128, 512)),         # [g*64+hh*16+l, 128q+32q+h]
    }
    out = nc.dram_tensor("score_part", [H, MH, S], F32, kind="ExternalOutput").ap()
    return ins, out


def _emit(tc: tile.TileContext, X, out):
    nc = tc.nc
    AL = mybir.AluOpType
    AF = mybir.ActivationFunctionType

    with ExitStack() as ctx:
        const = ctx.enter_context(tc.tile_pool(name="const", bufs=1))
        tabs = ctx.enter_context(tc.tile_pool(name="tabs", bufs=1))
        psum_pre = ctx.enter_context(tc.tile_pool(name="psum_pre", bufs=2, space="PSUM"))

        # ---- load raw inputs to SBUF (fp16, two HWDGE issue queues) ----
        ld_n = [0]

        def load(name, src=None, via_dve=False):
            if src is None:
                src = X[name]
            t = const.tile(list(src.shape), F16, name=f"sb_{name}")
            eng = nc.sync if ld_n[0] % 2 == 0 else nc.scalar
            ld_n[0] += 1
            eng.dma_start(out=t, in_=src)
            return t

        # ---- PE warm-up: ~7us of dummy matmuls so the HAM clock-gate opens
        # (K=8/8, 2.4 GHz) before the precompute/main matmuls start.  These
        # overlap the input-DMA phase, so they add no wall-clock.
        warm_w = const.tile([128, 128], BF16, name="warm_w")
        warm_r = const.tile([128, 512], BF16, name="warm_r")
        nc.vector.memset(warm_w, 0.0)
        nc.vector.memset(warm_r, 0.0)
        ps_warm = psum_pre.tile([128, 512], F32, name="ps_warm", tag="pre")
        for _ in range(N_WARMUP_MM):
            nc.tensor.matmul(out=ps_warm, lhsT=warm_w, rhs=warm_r,
                             start=True, stop=True)

        # critical-path loads first: Ap chain, then Acol chain, then the rest
        eposT = [load(f"eposT{g}", X["eposT"][g], via_dve=True) for g in range(2)]
        w1pe = [load(f"w1pe{g}", X["w1pe_blk"][g], via_dve=True) for g in range(2)]
        ebjT = load("ebjT", via_dve=True)
        ecjT = load("ecjT", via_dve=True)
        w1bj = load("w1bj_blk", via_dve=True)
        w1cj = load("w1cj_blk", via_dve=True)
        ohbc = load("oh_b_col", via_dve=True)
        ohcc = load("oh_c_col", via_dve=True)
        ebiT = load("ebiT", via_dve=True)
        eciT = load("eciT", via_dve=True)
        w1bi = load("w1bi_blk", via_dve=True)
        w1ci = load("w1ci_blk", via_dve=True)
        ohbr = load("oh_b_row", via_dve=True)
        ohcr = load("oh_c_row", via_dve=True)
        # stage-2/3 weights load directly as fp16
        w2h = [load(f"w2h_{g}", X["w2blk"][g]) for g in range(2)]
        w2f = tabs.tile([128, 128], F16, name="w2f")
        for g in range(2):
            nc.vector.tensor_copy(w2f[:, 64 * g:64 * (g + 1)], w2h[g])
        w3f = load("w3blk4")

        # ---- A-tables (all fp32 matmuls, accumulated in PSUM) ----
        # Ap[g]: [128=(hh,k), APW+4] fp16, col j <-> r_local = j-1
        Ap = [tabs.tile([128, APW + 4], F16, name=f"Ap{g}") for g in range(2)]
        for g in range(2):
            ps = psum_pre.tile([128, APW - 1], F32, name=f"ps_ap{g}", tag="pre")
            nc.tensor.matmul(out=ps, lhsT=w1pe[g], rhs=eposT[g][:, 1:APW],
                             start=True, stop=True)
            nc.scalar.copy(Ap[g][:, 2:APW + 1], ps)

        # T-tables: [v, (h,k)=256] fp32
        T = {}
        for nm, eT, wblk, P in (("bi", ebiT, w1bi, VB), ("ci", eciT, w1ci, VC),
                                ("bj", ebjT, w1bj, VB), ("cj", ecjT, w1cj, VC)):
            ps = psum_pre.tile([P, 256], F32, name=f"ps_t{nm}", tag="pre")
            nc.tensor.matmul(out=ps, lhsT=eT, rhs=wblk, start=True, stop=True)
            t = tabs.tile([P, 256], F16, name=f"T{nm}")
            nc.scalar.copy(t, ps)
            T[nm] = t

        # Arow[g]: [128=(hh,k), MH] fp32 ; Acol8[g]: [128, 8*S] fp16 (x8 copies)
        Arow = [tabs.tile([128, MH], F32, name=f"Arow{g}") for g in range(2)]
        Acol8 = [tabs.tile([128, 8 * S], F16, name=f"Acol8_{g}") for g in range(2)]
        for g in range(2):
            cs = slice(128 * g, 128 * (g + 1))
            ps = psum_pre.tile([128, MH], F32, name=f"ps_arow{g}", tag="pre")
            nc.tensor.matmul(out=ps, lhsT=T["bi"][:, cs], rhs=ohbr,
                             start=True, stop=False)
            nc.tensor.matmul(out=ps, lhsT=T["ci"][:, cs], rhs=ohcr,
                             start=False, stop=True)
            nc.vector.tensor_copy(Arow[g], ps)

            ps2 = psum_pre.tile([128, S], F32, name=f"ps_acol{g}", tag="pre")
            nc.tensor.matmul(out=ps2, lhsT=T["bj"][:, cs], rhs=ohbc,
                             start=True, stop=False)
            nc.tensor.matmul(out=ps2, lhsT=T["cj"][:, cs], rhs=ohcc,
                             start=False, stop=True)
            nc.scalar.copy(Acol8[g][:, 0:S], ps2)
            for w in (S, 2 * S, 4 * S):
                nc.vector.tensor_copy(Acol8[g][:, w:2 * w], Acol8[g][:, 0:w])

        # ---- main loop: 16 iterations x 8 m-values ----
        work = ctx.enter_context(tc.tile_pool(name="work", bufs=6))
        psum_m = ctx.enter_context(tc.tile_pool(name="psum_m", bufs=2, space="PSUM"))
        psum_o = ctx.enter_context(tc.tile_pool(name="psum_o", bufs=2, space="PSUM"))

        blk_n = [0]

        def bias_relu(dst, src, bias_col, eng):
            if eng == "dve":
                nc.vector.tensor_scalar(dst, src, bias_col, 0.0, AL.add, AL.max)
            else:
                nc.scalar.activation(out=dst, in_=src, func=AF.Relu,
                                     bias=bias_col, scale=1.0)

        ps3 = None
        for it in range(MH // 8):
            m0 = 8 * it
            tmps = []
            for g in range(2):
                # tmp = Ap[g][:, m+n'+2] + Acol  (sliding-window view of Ap)
                apg = Ap[g]
                apwin = bass.AP(
                    apg.tensor, apg.offset + (m0 + 2),
                    [list(apg.ap[0]), [1, 8], [1, S]],
                )
                tmp = work.tile([128, 2048], F16, name=f"tmp{g}_{it}", tag=f"tmp{g}")
                nc.vector.tensor_add(tmp, apwin, Acol8[g])
                tmps.append(tmp)

            for half in range(2):
                m4 = m0 + 4 * half
                ps2 = psum_m.tile([128, 1024], F32, name=f"ps2_{it}_{half}", tag="ps2")
                for g in range(2):
                    # h1 = max(tmp + Arow[:, m], 0) per m; bf16 out for stage 2
                    h1g = work.tile([128, 1024], F16, name=f"h1_{g}_{it}_{half}",
                                    tag=f"h1{g}")
                    # phase-aware routing: ACT is idle early and backlogged
                    # late, so front-load its share (same 16-block total).
                    n_act = 2 if it < 4 else (1 if it < 12 else 0)
                    eng = "act" if (2 * half + g) < n_act else "dve"
                    blk_n[0] += 1
                    for j in range(4):
                        o = 1024 * half + S * j
                        bias_relu(h1g[:, S * j:S * (j + 1)], tmps[g][:, o:o + S],
                                  Arow[g][:, m4 + j:m4 + j + 1], eng)
                    for c in range(2):
                        nc.tensor.matmul(
                            out=ps2[64 * g:64 * (g + 1), 512 * c:512 * (c + 1)],
                            lhsT=w2f[:, 64 * g:64 * (g + 1)],
                            rhs=h1g[:, 512 * c:512 * (c + 1)],
                            start=True, stop=True,
                        )

                h2 = work.tile([128, 1024], F16, name=f"h2_{it}_{half}", tag="h2")
                if it >= 12:
                    # tail: ACT is backlogged, DVE already idle -- swap engines
                    nc.vector.tensor_scalar(h2, ps2, 0.0, None, AL.max)
                else:
                    nc.scalar.activation(out=h2, in_=ps2, func=AF.Relu)

                # stage 3: pack four [8,512] score halves into one PSUM bank
                # via zero-padded lhsT variants; one copy to SBUF; DMA out.
                for c in range(2):
                    q = (4 * it + 2 * half + c) % 4
                    if q == 0:
                        ps3 = psum_o.tile([128, 512], F32, name=f"ps3_{it}", tag="ps3")
                    nc.tensor.matmul(out=ps3, lhsT=w3f[:, 128 * q:128 * (q + 1)],
                                     rhs=h2[:, 512 * c:512 * (c + 1)],
                                     start=(q == 0), stop=(q == 3))
                    if q == 3:
                        sc = work.tile([128, 512], F32, name=f"sc_{it}", tag="sc")
                        if it >= 12:
                            nc.vector.tensor_copy(sc, ps3)
                        else:
                            nc.scalar.copy(sc, ps3)
                        mbase = m4 + 2 * c - 6  # first m of the 4 packed halves
                        for qq in range(4):
                            # drain on two DMA queues in parallel (HWDGE + SWDGE)
                            dma_eng = nc.sync if qq % 2 == 0 else nc.gpsimd
                            dma_eng.dma_start(
                                out=out[:, mbase + 2 * qq:mbase + 2 * qq + 2, :],
                                in_=sc[32 * qq:32 * qq + 8, :],
                            )


_PROGRAM = None


def _get_program():
    global _PROGRAM
    if _PROGRAM is None:
        nc = bacc.Bacc("TRN2", debug=False, num_devices=8)
        ins, out = _declare_io(nc)
        with tile.TileContext(nc) as tc:
            _emit(tc, ins, out)
        nc.compile()
        _PROGRAM = nc
    return _PROGRAM


def _build_in_maps(inputs):
    b_seq = np.asarray(inputs["b_seq"]).astype(np.int64)
    c_seq = np.asarray(inputs["c_seq"]).astype(np.int64)
    e_pos = np.asarray(inputs["e_pos"]).astype(np.float32)   # (512, 8, 32)
    e_bi = np.asarray(inputs["e_bi"]).astype(np.float32)     # (11, 8, 16)
    e_bj = np.asarray(inputs["e_bj"]).astype(np.float32)
    e_ci = np.asarray(inputs["e_ci"]).astype(np.float32)     # (102, 8, 16)
    e_cj = np.asarray(inputs["e_cj"]).astype(np.float32)
    w1 = np.asarray(inputs["w1_e"]).astype(np.float32)       # (96, 32, 8)
    w2 = np.asarray(inputs["w2_e"]).astype(np.float32)       # (32, 16, 8)
    w3 = np.asarray(inputs["w3_e"]).astype(np.float32)       # (16, 8)

    C = lambda a: np.ascontiguousarray(a.astype(np.float16))

    # [h*32+d, r] and [h*16+d, v] transposed table layouts
    eposT_full = C(e_pos.transpose(1, 2, 0).reshape(H * 32, 2 * S))
    ebiT = C(e_bi.transpose(1, 2, 0).reshape(128, VB))
    ebjT = C(e_bj.transpose(1, 2, 0).reshape(128, VB))
    eciT = C(e_ci.transpose(1, 2, 0).reshape(128, VC))
    ecjT = C(e_cj.transpose(1, 2, 0).reshape(128, VC))

    # block-diagonal w1 pieces
    w1pe_blk = np.zeros((2, 128, 128), np.float16)
    for g in range(2):
        for hh in range(4):
            w1pe_blk[g, 32 * hh:32 * (hh + 1), 32 * hh:32 * (hh + 1)] = w1[0:32, :, 4 * g + hh]

    def blk16(w1rows):  # (16, 32, 8) -> [h*16+d, h*32+k]
        m = np.zeros((128, 256), np.float16)
        for h in range(H):
            m[16 * h:16 * (h + 1), 32 * h:32 * (h + 1)] = w1rows[:, :, h]
        return m

    w1bi_blk = blk16(w1[32:48])
    w1bj_blk = blk16(w1[48:64])
    w1ci_blk = blk16(w1[64:80])
    w1cj_blk = blk16(w1[80:96])

    w2blk = np.zeros((2, 128, 64), np.float16)
    for g in range(2):
        for hh in range(4):
            w2blk[g, 32 * hh:32 * (hh + 1), 16 * hh:16 * (hh + 1)] = w2[:, :, 4 * g + hh]
    w3blk4 = np.zeros((128, 512), np.float16)
    for q in range(4):
        for h in range(H):
            w3blk4[16 * h:16 * (h + 1), 128 * q + 32 * q + h] = w3[:, h]

    shared = {
        "ebiT": ebiT, "ebjT": ebjT, "eciT": eciT, "ecjT": ecjT,
        "w1pe_blk": w1pe_blk, "w1bi_blk": w1bi_blk, "w1bj_blk": w1bj_blk,
        "w1ci_blk": w1ci_blk, "w1cj_blk": w1cj_blk,
        "w2blk": w2blk, "w3blk4": w3blk4,
    }

    def onehot(seq_slice, nv):
        oh = np.zeros((nv, len(seq_slice)), np.float16)
        oh[seq_slice, np.arange(len(seq_slice))] = 1.0
        return oh

    in_maps = []
    for core in range(8):
        b, half = core // 2, core % 2
        m_off = half * MH
        im = dict(shared)
        im["eposT"] = C(eposT_full[:, m_off:m_off + APW].reshape(2, 128, APW))
        im["oh_b_row"] = onehot(b_seq[b, m_off:m_off + MH], VB)
        im["oh_c_row"] = onehot(c_seq[b, m_off:m_off + MH], VC)
        im["oh_b_col"] = onehot(b_seq[b, ::-1], VB)
        im["oh_c_col"] = onehot(c_seq[b, ::-1], VC)
        in_maps.append(im)
    return in_maps


def _assemble(core_outs):
    score = np.empty((B, H, S, S), np.float32)
    for core in range(8):
        b, half = core // 2, core % 2
        part = core_outs[core]["score_part"]        # [H, MH, S] (n-reversed)
        score[b, :, half * MH:(half + 1) * MH, :] = part[:, :, ::-1]
    return score


def kernel(**inputs) -> np.ndarray:
    in_maps = _build_in_maps(inputs)
    nc = _get_program()

    if os.environ.get("BASSK_SIM"):
        from concourse.bass_interp import CoreSim
        outs = []
        for core in [int(x) for x in os.environ["BASSK_SIM"].split(",")]:
            sim = CoreSim(nc, trace=False)
            for k, v in in_maps[core].items():
                sim.tensor(k)[:] = v
            sim.simulate(check_with_hw=False)
            outs.append((core, {"score_part": sim.tensor("score_part").copy()}))
        score = np.zeros((B, H, S, S), np.float32)
        for core, o in outs:
            b, half = core // 2, core % 2
            score[b, :, half * MH:(half + 1) * MH, :] = o["score_part"][:, :, ::-1]
        return score

    res = run_bass_kernel_spmd(nc, in_maps, core_ids=list(range(8)))
    return _assemble(res.results)


# revision 39
# speedup vs baseline: 1.0507x; 1.0507x over previous
"""Trainium2 Bass kernel for nn_DisAttLayer (disentangled-attention bias MLP).

Math (reference):
    e[b,m,n,h,:] = concat(pe[m-n+S], bi[b,m], bj[b,n], ci[b,m], cj[b,n])  (96)
    h1 = relu(e @ w1[:, :, h])     (96->32, per head)
    h2 = relu(h1 @ w2[:, :, h])    (32->16)
    score[b,h,m,n] = h2 @ w3[:, h] (16->1)

Key factorization: layer 1 is linear in the concat, so
    h1pre[b,m,n,h,k] = Ap[m-n+S,h,k] + Arow[b,m,h,k] + Acol[b,n,h,k]
where Ap/Arow/Acol are tiny per-table transforms (computed on-device from the
raw embedding tables and w1).  With the free axis taken as n' = 255-n, the
relative-position gather Ap[m-n+S] becomes a contiguous slice of a 384-wide
table, so no gather is needed at all.  Only layers 2+3 touch the full
(B,S,S,H) volume.

Sharding: 8 cores = batch b (4) x query-half m (2).  All per-core variation
is moved into the input data (pre-shifted e_pos slice, per-core one-hot
index masks), so a single SPMD program serves all cores.

Host does layout only (transpose/reshape/zero-pad/slicing of raw inputs and
integer->one-hot relabeling of the index sequences); every floating-point
multiply/add of the model runs on device.
"""

import os
from contextlib import ExitStack

import numpy as np

import concourse.bacc as bacc
import concourse.bass as bass
import concourse.tile as tile
from concourse import mybir
from concourse.bass_utils import run_bass_kernel_spmd

S = 256
H = 8
B = 4
MH = 128          # m-values per core
VB = 11           # e_bi / e_bj rows  (N_MB + 1)
VC = 102          # e_ci / e_cj rows  (N_C + 2)
APW = MH + S      # 384: width of the per-core shifted e_pos slice

F32 = mybir.dt.float32
F16 = mybir.dt.float16
BF16 = mybir.dt.bfloat16

# engine routing for the per-(m, head-group) bias+relu blocks (structural:
# one op per m since the bias is a per-partition scalar).  Routed in whole
# 4-op blocks to minimize cross-engine semaphore traffic; DVE is ~2x faster
# per op than ACT but ACT has idle capacity.
TS_ROUTE = ("dve", "dve", "act", "dve", "dve", "act", "dve", "dve")
N_WARMUP_MM = 12


def _declare_io(nc):
    def inp(name, shape):
        return nc.dram_tensor(name, list(shape), F16, kind="ExternalInput").ap()

    ins = {
        "eposT": inp("eposT", (2, 128, APW)),       # [g][hh*32+d, r_local]
        "ebiT": inp("ebiT", (128, VB)),             # [h*16+d, v]
        "ebjT": inp("ebjT", (128, VB)),
        "eciT": inp("eciT", (128, VC)),
        "ecjT": inp("ecjT", (128, VC)),
        "w1pe_blk": inp("w1pe_blk", (2, 128, 128)),  # [g][hh*32+d, hh*32+k]
        "w1bi_blk": inp("w1bi_blk", (128, 256)),     # [h*16+d, h*32+k]
        "w1bj_blk": inp("w1bj_blk", (128, 256)),
        "w1ci_blk": inp("w1ci_blk", (128, 256)),
        "w1cj_blk": inp("w1cj_blk", (128, 256)),
        "oh_b_row": inp("oh_b_row", (VB, MH)),
        "oh_c_row": inp("oh_c_row", (VC, MH)),
        "oh_b_col": inp("oh_b_col", (VB, S)),        # n-reversed
        "oh_c_col": inp("oh_c_col", (VC, S)),
        "w2blk": inp("w2blk", (2, 128, 64)),         # [g][hh*32+k, hh*16+l]
        # 4 zero-padded variants; variant q maps head h -> out partition 32q+h
        "w3blk4": inp("w3blk4", (128, 512)),         # [g*64+hh*16+l, 128q+32q+h]
    }
    out = nc.dram_tensor("score_part", [H, MH, S], F32, kind="ExternalOutput").ap()
    return ins, out


def _emit(tc: tile.TileContext, X, out):
    nc = tc.nc
    AL = mybir.AluOpType
    AF = mybir.ActivationFunctionType

    with ExitStack() as ctx:
        const = ctx.enter_context(tc.tile_pool(name="const", bufs=1))
        tabs = ctx.enter_context(tc.tile_pool(name="tabs", bufs=1))
        psum_pre = ctx.enter_context(tc.tile_pool(name="psum_pre", bufs=2, space="PSUM"))

        # ---- load raw inputs to SBUF (fp16, two HWDGE issue queues) ----
        ld_n = [0]

        def load(name, src=None, via_dve=False):
            if src is None:
                src = X[name]
            t = const.tile(list(src.shape), F16, name=f"sb_{name}")
            eng = nc.sync if ld_n[0] % 2 == 0 else nc.scalar
            ld_n[0] += 1
            eng.dma_start(out=t, in_=src)
            return t

        # ---- PE warm-up: ~7us of dummy matmuls so the HAM clock-gate opens
        # (K=8/8, 2.4 GHz) before the precompute/main matmuls start.  These
        # overlap the input-DMA phase, so they add no wall-clock.
        warm_w = const.tile([128, 128], BF16, name="warm_w")
        warm_r = const.tile([128, 512], BF16, name="warm_r")
        nc.vector.memset(warm_w, 0.0)
        nc.vector.memset(warm_r, 0.0)
        ps_warm = psum_pre.tile([128, 512], F32, name="ps_warm", tag="pre")
        for _ in range(N_WARMUP_MM):
            nc.tensor.matmul(out=ps_warm, lhsT=warm_w, rhs=warm_r,
                             start=True, stop=True)

        # critical-path loads first: Ap chain, then Acol chain, then the rest
        eposT = [load(f"eposT{g}", X["eposT"][g], via_dve=True) for g in range(2)]
        w1pe = [load(f"w1pe{g}", X["w1pe_blk"][g], via_dve=True) for g in range(2)]
        ebjT = load("ebjT", via_dve=True)
        ecjT = load("ecjT", via_dve=True)
        w1bj = load("w1bj_blk", via_dve=True)
        w1cj = load("w1cj_blk", via_dve=True)
        ohbc = load("oh_b_col", via_dve=True)
        ohcc = load("oh_c_col", via_dve=True)
        ebiT = load("ebiT", via_dve=True)
        eciT = load("eciT", via_dve=True)
        w1bi = load("w1bi_blk", via_dve=True)
        w1ci = load("w1ci_blk", via_dve=True)
        ohbr = load("oh_b_row", via_dve=True)
        ohcr = load("oh_c_row", via_dve=True)
        # stage-2/3 weights load directly as fp16
        w2h = [load(f"w2h_{g}", X["w2blk"][g]) for g in range(2)]
        w2f = tabs.tile([128, 128], F16, name="w2f")
        for g in range(2):
            nc.vector.tensor_copy(w2f[:, 64 * g:64 * (g + 1)], w2h[g])
        w3f = load("w3blk4")

        # ---- A-tables (all fp32 matmuls, accumulated in PSUM) ----
        # Ap[g]: [128=(hh,k), APW+4] fp16, col j <-> r_local = j-1
        Ap = [tabs.tile([128, APW + 4], F16, name=f"Ap{g}") for g in range(2)]
        for g in range(2):
            ps = psum_pre.tile([128, APW - 1], F32, name=f"ps_ap{g}", tag="pre")
            nc.tensor.matmul(out=ps, lhsT=w1pe[g], rhs=eposT[g][:, 1:APW],
                             start=True, stop=True)
            nc.scalar.copy(Ap[g][:, 2:APW + 1], ps)

        # T-tables: [v, (h,k)=256] fp32
        T = {}
        for nm, eT, wblk, P in (("bi", ebiT, w1bi, VB), ("ci", eciT, w1ci, VC),
                                ("bj", ebjT, w1bj, VB), ("cj", ecjT, w1cj, VC)):
            ps = psum_pre.tile([P, 256], F32, name=f"ps_t{nm}", tag="pre")
            nc.tensor.matmul(out=ps, lhsT=eT, rhs=wblk, start=True, stop=True)
            t = tabs.tile([P, 256], F16, name=f"T{nm}")
            nc.scalar.copy(t, ps)
            T[nm] = t

        # Arow[g]: [128=(hh,k), MH] fp32 ; Acol8[g]: [128, 8*S] fp16 (x8 copies)
        Arow = [tabs.tile([128, MH], F32, name=f"Arow{g}") for g in range(2)]
        Acol8 = [tabs.tile([128, 8 * S], F16, name=f"Acol8_{g}") for g in range(2)]
        for g in range(2):
            cs = slice(128 * g, 128 * (g + 1))
            ps = psum_pre.tile([128, MH], F32, name=f"ps_arow{g}", tag="pre")
            nc.tensor.matmul(out=ps, lhsT=T["bi"][:, cs], rhs=ohbr,
                             start=True, stop=False)
            nc.tensor.matmul(out=ps, lhsT=T["ci"][:, cs], rhs=ohcr,
                             start=False, stop=True)
            nc.vector.tensor_copy(Arow[g], ps)

            ps2 = psum_pre.tile([128, S], F32, name=f"ps_acol{g}", tag="pre")
            nc.tensor.matmul(out=ps2, lhsT=T["bj"][:, cs], rhs=ohbc,
                             start=True, stop=False)
            nc.tensor.matmul(out=ps2, lhsT=T["cj"][:, cs], rhs=ohcc,
                             start=False, stop=True)
            nc.scalar.copy(Acol8[g][:, 0:S], ps2)
            for w in (S, 2 * S, 4 * S):
                nc.vector.tensor_copy(Acol8[g][:, w:2 * w], Acol8[g][:, 0:w])

        # ---- main loop: 16 iterations x 8 m-values ----
        work = ctx.enter_context(tc.tile_pool(name="work", bufs=6))
        psum_m = ctx.enter_context(tc.tile_pool(name="psum_m", bufs=2, space="PSUM"))
        psum_o = ctx.enter_context(tc.tile_pool(name="psum_o", bufs=2, space="PSUM"))

        blk_n = [0]

        def bias_relu(dst, src, bias_col, eng):
            if eng == "dve":
                nc.vector.tensor_scalar(dst, src, bias_col, 0.0, AL.add, AL.max)
            else:
                nc.scalar.activation(out=dst, in_=src, func=AF.Relu,
                                     bias=bias_col, scale=1.0)

        ps3 = None
        for it in range(MH // 8):
            m0 = 8 * it
            tmps = []
            for g in range(2):
                # tmp = Ap[g][:, m+n'+2] + Acol  (sliding-window view of Ap)
                apg = Ap[g]
                apwin = bass.AP(
                    apg.tensor, apg.offset + (m0 + 2),
                    [list(apg.ap[0]), [1, 8], [1, S]],
                )
                tmp = work.tile([128, 2048], F16, name=f"tmp{g}_{it}", tag=f"tmp{g}")
                nc.vector.tensor_add(tmp, apwin, Acol8[g])
                tmps.append(tmp)

            for half in range(2):
                m4 = m0 + 4 * half
                ps2 = psum_m.tile([128, 1024], F32, name=f"ps2_{it}_{half}", tag="ps2")
                for g in range(2):
                    # h1 = max(tmp + Arow[:, m], 0) per m; bf16 out for stage 2
                    h1g = work.tile([128, 1024], F16, name=f"h1_{g}_{it}_{half}",
                                    tag=f"h1{g}")
                    eng = TS_ROUTE[blk_n[0] % len(TS_ROUTE)]
                    blk_n[0] += 1
                    for j in range(4):
                        o = 1024 * half + S * j
                        bias_relu(h1g[:, S * j:S * (j + 1)], tmps[g][:, o:o + S],
                                  Arow[g][:, m4 + j:m4 + j + 1], eng)
                    for c in range(2):
                        nc.tensor.matmul(
                            out=ps2[64 * g:64 * (g + 1), 512 * c:512 * (c + 1)],
                            lhsT=w2f[:, 64 * g:64 * (g + 1)],
                            rhs=h1g[:, 512 * c:512 * (c + 1)],
                            start=True, stop=True,
                        )

                h2 = work.tile([128, 1024], F16, name=f"h2_{it}_{half}", tag="h2")
                nc.scalar.activation(out=h2, in_=ps2, func=AF.Relu)

                # stage 3: pack four [8,512] score halves into one PSUM bank
                # via zero-padded lhsT variants; one copy to SBUF; DMA out.
                for c in range(2):
                    q = (4 * it + 2 * half + c) % 4
                    if q == 0:
                        ps3 = psum_o.tile([128, 512], F32, name=f"ps3_{it}", tag="ps3")
                    nc.tensor.matmul(out=ps3, lhsT=w3f[:, 128 * q:128 * (q + 1)],
                                     rhs=h2[:, 512 * c:512 * (c + 1)],
                                     start=(q == 0), stop=(q == 3))
                    if q == 3:
                        sc = work.tile([128, 512], F32, name=f"sc_{it}", tag="sc")
                        nc.scalar.copy(sc, ps3)
                        mbase = m4 + 2 * c - 6  # first m of the 4 packed halves
                        for qq in range(4):
                            # drain on two DMA queues in parallel (HWDGE + SWDGE)
                            dma_eng = nc.sync if qq % 2 == 0 else nc.gpsimd
                            dma_eng.dma_start(
                                out=out[:, mbase + 2 * qq:mbase + 2 * qq + 2, :],
                                in_=sc[32 * qq:32 * qq + 8, :],
                            )


_PROGRAM = None


def _get_program():
    global _PROGRAM
    if _PROGRAM is None:
        nc = bacc.Bacc("TRN2", debug=False, num_devices=8)
        ins, out = _declare_io(nc)
        with tile.TileContext(nc) as tc:
            _emit(tc, ins, out)
        nc.compile()
        _PROGRAM = nc
    return _PROGRAM


def _build_in_maps(inputs):
    b_seq = np.asarray(inputs["b_seq"]).astype(np.int64)
    c_seq = np.asarray(inputs["c_seq"]).astype(np.int64)
    e_pos = np.asarray(inputs["e_pos"]).astype(np.float32)   # (512, 8, 32)
    e_bi = np.asarray(inputs["e_bi"]).astype(np.float32)     # (11, 8, 16)
    e_bj = np.asarray(inputs["e_bj"]).astype(np.float32)
    e_ci = np.asarray(inputs["e_ci"]).astype(np.float32)     # (102, 8, 16)
    e_cj = np.asarray(inputs["e_cj"]).astype(np.float32)
    w1 = np.asarray(inputs["w1_e"]).astype(np.float32)       # (96, 32, 8)
    w2 = np.asarray(inputs["w2_e"]).astype(np.float32)       # (32, 16, 8)
    w3 = np.asarray(inputs["w3_e"]).astype(np.float32)       # (16, 8)

    C = lambda a: np.ascontiguousarray(a.astype(np.float16))

    # [h*32+d, r] and [h*16+d, v] transposed table layouts
    eposT_full = C(e_pos.transpose(1, 2, 0).reshape(H * 32, 2 * S))
    ebiT = C(e_bi.transpose(1, 2, 0).reshape(128, VB))
    ebjT = C(e_bj.transpose(1, 2, 0).reshape(128, VB))
    eciT = C(e_ci.transpose(1, 2, 0).reshape(128, VC))
    ecjT = C(e_cj.transpose(1, 2, 0).reshape(128, VC))

    # block-diagonal w1 pieces
    w1pe_blk = np.zeros((2, 128, 128), np.float16)
    for g in range(2):
        for hh in range(4):
            w1pe_blk[g, 32 * hh:32 * (hh + 1), 32 * hh:32 * (hh + 1)] = w1[0:32, :, 4 * g + hh]

    def blk16(w1rows):  # (16, 32, 8) -> [h*16+d, h*32+k]
        m = np.zeros((128, 256), np.float16)
        for h in range(H):
            m[16 * h:16 * (h + 1), 32 * h:32 * (h + 1)] = w1rows[:, :, h]
        return m

    w1bi_blk = blk16(w1[32:48])
    w1bj_blk = blk16(w1[48:64])
    w1ci_blk = blk16(w1[64:80])
    w1cj_blk = blk16(w1[80:96])

    w2blk = np.zeros((2, 128, 64), np.float16)
    for g in range(2):
        for hh in range(4):
            w2blk[g, 32 * hh:32 * (hh + 1), 16 * hh:16 * (hh + 1)] = w2[:, :, 4 * g + hh]
    w3blk4 = np.zeros((128, 512), np.float16)
    for q in range(4):
        for h in range(H):
            w3blk4[16 * h:16 * (h + 1), 128 * q + 32 * q + h] = w3[:, h]

    shared = {
        "ebiT": ebiT, "ebjT": ebjT, "eciT": eciT, "ecjT": ecjT,
        "w1pe_blk": w1pe_blk, "w1bi_blk": w1bi_blk, "w1bj_blk": w1bj_blk,
        "w1ci_blk": w1ci_blk, "w1cj_blk": w1cj_blk,
        "w2blk": w2blk, "w3blk4": w3blk4,
    }

    def onehot(seq_slice, nv):
        oh = np.zeros((nv, len(seq_slice)), np.float16)
        oh[seq_slice, np.arange(len(seq_slice))] = 1.0
        return oh

    in_maps = []
    for core in range(8):
        b, half = core // 2, core % 2
        m_off = half * MH
        im = dict(shared)
        im["eposT"] = C(eposT_full[:, m_off:m_off + APW].reshape(2, 128, APW))
        im["oh_b_row"] = onehot(b_seq[b, m_off:m_off + MH], VB)
        im["oh_c_row"] = onehot(c_seq[b, m_off:m_off + MH], VC)
        im["oh_b_col"] = onehot(b_seq[b, ::-1], VB)
        im["oh_c_col"] = onehot(c_seq[b, ::-1], VC)
        in_maps.append(im)
    return in_maps


def _assemble(core_outs):
    score = np.empty((B, H, S, S), np.float32)
    for core in range(8):
        b, half = core // 2, core % 2
        part = core_outs[core]["score_part"]        # [H, MH, S] (n-reversed)
        score[b, :, half * MH:(half + 1) * MH, :] = part[:, :, ::-1]
    return score


def kernel(**inputs) -> np.ndarray:
    in_maps = _build_in_maps(inputs)
    nc = _get_program()

    if os.environ.get("BASSK_SIM"):
        from concourse.bass_interp import CoreSim
        outs = []
        for core in [int(x) for x in os.environ["BASSK_SIM"].split(",")]:
            sim = CoreSim(nc, trace=False)
            for k, v in in_maps[core].items():
                sim.tensor(k)[:] = v
            sim.simulate(check_with_hw=False)
            outs.append((core, {"score_part": sim.tensor("score_part").copy()}))
        score = np.zeros((B, H, S, S), np.float32)
        for core, o in outs:
            b, half = core // 2, core % 2
            score[b, :, half * MH:(half + 1) * MH, :] = o["score_part"][:, :, ::-1]
        return score

    res = run_bass_kernel_spmd(nc, in_maps, core_ids=list(range(8)))
    return _assemble(res.results)
